# revision 26
# baseline (speedup 1.0000x reference)
"""Axial attention (no softmax) on 8 TRN2 NeuronCores.

Problem: x (8, 64, 64, 1024) fp32; two self-attentions (16 heads, no
softmax, scale d**-0.5) along the H axis (w_qkv0/w_out0) and the W axis
(w_qkv1/w_out1); output is their sum.

Sharding: data-parallel over batch B=8 -> one batch slab per core,
weights replicated. Each core computes both axial passes for its slab;
no collectives.

Per-core kernel structure (all matmuls bf16, fp32 PSUM accumulate):
  x is pre-transposed on the HOST into two [D, NT] bf16 layouts (xth
  w-major for the H pass, xtw h-major for the W pass) -- no PE
  transposes on device. 16 chunks of 8 sequences (CH=512 tokens) are
  processed as a software pipeline; per iteration k:

    v(k) [+ A packs k-1, 4..7]    v[tb] = x @ Wv, natural layout
    qkT(k) [+ O packs k-1, 0..7]  qkT[m] = Wqk[:,m].T @ xT
    y(k-1) [+ A packs k, 0..3]    y = OT.T @ Wout, DMA out

  The 64x64 attention packs (A^T = kT.T @ qT and O^T = v.T @ A^T, 4-way
  tile_position packing) are woven one pack (~0.25us) per ~3.5us of fat
  matmuls: the PE array never idles, the HAM clock gate stays at 8/8,
  and the packs' PSUM->SBUF copies (the att stage's real cost, ~4.2MB
  per chunk on DVE+ACT) overlap fat PE work instead of serializing.

  Pass H (chunks 0-7) writes out directly; pass W (chunks 8-15)
  gpsimd-DMA-accumulates (out = oh + ow).
"""

import numpy as np
import ml_dtypes
from contextlib import ExitStack

from concourse.bass_utils import run_bass_kernel_spmd
from concourse import bacc, mybir, tile
from concourse.masks import make_identity

BF16 = mybir.dt.bfloat16
F32 = mybir.dt.float32

B = 8
D = 1024
NT = 4096
CH = 512
NCHUNK = NT // CH  # 8 per pass
KB = D // 128      # 8 contraction blocks
SCALE = 1.0 / 32.0

_BUILD_CACHE = {}
STAGE_MAP = {}


class _TensorProxy:
    def __init__(self, te):
        self._te = te
        self.stage = "?"

    def matmul(self, *a, **kw):
        r = self._te.matmul(*a, **kw)
        STAGE_MAP[r.ins.name] = self.stage
        return r


def build():
    key = "pipe"
    if key in _BUILD_CACHE:
        return _BUILD_CACHE[key]

    nc = bacc.Bacc("TRN2", target_bir_lowering=False, debug=False)
    xth = nc.dram_tensor("xth", [D, NT], BF16, kind="ExternalInput")
    xtw = nc.dram_tensor("xtw", [D, NT], BF16, kind="ExternalInput")
    wqk = [nc.dram_tensor(f"wqk{p}", [D, 2 * D], BF16, kind="ExternalInput")
           for p in range(2)]
    wv = [nc.dram_tensor(f"wv{p}", [D, D], BF16, kind="ExternalInput")
          for p in range(2)]
    wo = [nc.dram_tensor(f"wo{p}", [D, D], BF16, kind="ExternalInput")
          for p in range(2)]
    out = nc.dram_tensor("out", [NT, D], F32, kind="ExternalOutput")
    og = out.rearrange("(h w) d -> w h d", w=64)

    NIT = 2 * NCHUNK  # 16 pipeline iterations; chunk k: pass k//8, c k%8

    with tile.TileContext(nc) as tc, ExitStack() as ctx:
        def pool(name, bufs, space="SBUF"):
            return ctx.enter_context(
                tc.tile_pool(name=name, bufs=bufs, space=space))

        p_id = pool("ident", 1)
        p_wqk = pool("wqk", 12)
        p_wv = pool("wv", 8)
        p_wo = pool("wo", 10)
        p_xt = pool("xt", 16)
        p_qkt = pool("qkt", 26)
        p_v = pool("v", 10)
        p_sa = pool("sa", 36)
        p_ot = pool("ot", 16)
        p_y = pool("y", 4)
        ps_big = pool("psb", 3, "PSUM")
        ps_att = pool("psatt", 5, "PSUM")

        te = _TensorProxy(nc.tensor)
        ident = p_id.tile([128, 128], BF16, name="ident")
        make_identity(nc, ident)

        dq = (nc.sync, nc.scalar, nc.gpsimd)

        # per-chunk state, indexed by iteration number
        st = {}          # k -> dict(xt, qkt, v_t, sa, ot, ybufs)
        wts = {}         # p -> dict(wqk, wv, wo)

        dq2 = (nc.sync, nc.gpsimd)

        def fetch_xt(k):
            # sync+gpsimd only: a DMA issue costs ~700ns of engine time,
            # and the scalar engine's queue must stay clear for the
            # attention packs' PSUM->SBUF copies
            p, c = k // NCHUNK, k % NCHUNK
            xsrc = xth if p == 0 else xtw
            ts = []
            for kk in range(KB):
                t = p_xt.tile([128, CH], BF16, tag="xt", name=f"xt_{k}_{kk}")
                dq2[(k * KB + kk) % 2].dma_start(
                    t[:], xsrc[kk * 128:(kk + 1) * 128, c * CH:(c + 1) * CH])
                ts.append(t)
            return ts

        def fetch_weights(p, interleave_xt0=False):
            # startup fetch uses all 3 queues (scalar is empty then);
            # the mid-kernel pass-1 refetch keeps off the scalar engine
            q = dq if interleave_xt0 else dq2
            nq = len(q)
            w = {"wqk": [], "wv": [], "wo": []}
            xts = []
            for k in range(KB):
                if interleave_xt0:
                    t = p_xt.tile([128, CH], BF16, tag="xt",
                                  name=f"xt_0_{k}")
                    q[k % nq].dma_start(
                        t[:], xth[k * 128:(k + 1) * 128, 0:CH])
                    xts.append(t)
                t = p_wqk.tile([128, 2 * D], BF16, tag="wqk",
                               name=f"wqk_{p}_{k}")
                q[k % nq].dma_start(t[:], wqk[p][k * 128:(k + 1) * 128, :])
                w["wqk"].append(t)
            for k in range(KB):
                t = p_wv.tile([128, D], BF16, tag="wv", name=f"wv_{p}_{k}")
                q[k % nq].dma_start(t[:], wv[p][k * 128:(k + 1) * 128, :])
                w["wv"].append(t)
            for k in range(KB):
                t = p_wo.tile([128, D], BF16, tag="wo", name=f"wo_{p}_{k}")
                q[(k + 1) % nq].dma_start(t[:], wo[p][k * 128:(k + 1) * 128, :])
                w["wo"].append(t)
            wts[p] = w
            return xts

        def cp(eng, dst, src_):
            if eng is nc.scalar:
                eng.copy(dst, src_)
            else:
                eng.tensor_copy(dst, src_)

        def emit_A(k, j):
            te.stage = "attA"
            s = st[k]
            kq = s["qkt"][8 + j]
            qq = s["qkt"][j]
            paE = ps_att.tile([128, 256], F32, tag="att", name=f"paE_{k}_{j}")
            paO = ps_att.tile([128, 256], F32, tag="att", name=f"paO_{k}_{j}")
            for sq in range(8):
                rp = (sq % 2) * 64
                fc = (sq // 2) * 64
                ssl = slice(sq * 64, (sq + 1) * 64)
                te.matmul(paE[rp:rp + 64, fc:fc + 64],
                          lhsT=kq[0:64, ssl], rhs=qq[0:64, ssl],
                          start=True, stop=True, tile_position=(0, rp))
                te.matmul(paO[rp:rp + 64, fc:fc + 64],
                          lhsT=kq[64:128, ssl], rhs=qq[64:128, ssl],
                          start=True, stop=True, tile_position=(64, rp))
            saE = p_sa.tile([128, 256], BF16, tag="sa", name=f"saE_{k}_{j}")
            saO = p_sa.tile([128, 256], BF16, tag="sa", name=f"saO_{k}_{j}")
            cp(nc.vector, saE[:], paE[:])
            cp(nc.scalar, saO[:], paO[:])
            s["sa"][j] = (saE, saO)

        def emit_O(k, j):
            te.stage = "attO"
            s = st[k]
            saE, saO = s["sa"][j]
            poS0 = ps_att.tile([128, 256], F32, tag="att", name=f"poS0_{k}_{j}")
            poS1 = ps_att.tile([128, 256], F32, tag="att", name=f"poS1_{k}_{j}")
            h0 = slice((2 * j) * 64, (2 * j + 1) * 64)
            h1 = slice((2 * j + 1) * 64, (2 * j + 2) * 64)
            for sq in range(8):
                rv = (sq % 2) * 64
                fc = (sq // 2) * 64
                vv = s["v_t"][sq // 2]
                dst = poS0 if sq % 2 == 0 else poS1
                te.matmul(dst[0:64, fc:fc + 64],
                          lhsT=vv[rv:rv + 64, h0],
                          rhs=saE[rv:rv + 64, fc:fc + 64],
                          start=True, stop=True, tile_position=(rv, 0))
                te.matmul(dst[64:128, fc:fc + 64],
                          lhsT=vv[rv:rv + 64, h1],
                          rhs=saO[rv:rv + 64, fc:fc + 64],
                          start=True, stop=True, tile_position=(rv, 64))
            otv = s["ot"][j].rearrange("p (s2 par t) -> p par s2 t",
                                       par=2, t=64)
            po0v = poS0.rearrange("p (s2 t) -> p s2 t", t=64)
            po1v = poS1.rearrange("p (s2 t) -> p s2 t", t=64)
            cp(nc.vector, otv[:, 0], po0v)
            cp(nc.scalar if j % 4 else nc.vector, otv[:, 1], po1v)

        def emit_v_stage(k, opacks):
            # v groups with previous-chunk O packs woven after every 2nd
            # group (same proven one-pack-per-3.5us density)
            p = k // NCHUNK
            s = st[k]
            wv_t = wts[p]["wv"]
            oi = 0
            for g in range(8):
                tb, n2 = g // 2, g % 2
                te.stage = "v"
                pv = ps_big.tile([128, CH], F32, tag="big",
                                 name=f"pv_{k}_{tb}_{n2}")
                for kk in range(KB):
                    te.matmul(pv[:],
                              lhsT=s["xt"][kk][:, tb * 128:(tb + 1) * 128],
                              rhs=wv_t[kk][:, n2 * 512:(n2 + 1) * 512],
                              start=(kk == 0), stop=(kk == KB - 1))
                nc.vector.tensor_copy(
                    s["v_t"][tb][:, n2 * 512:(n2 + 1) * 512], pv[:])
                if g % 2 == 1 and oi < len(opacks):
                    emit_O(*opacks[oi])
                    oi += 1
                    te.stage = "v"

        def emit_qkT_stage(k, opacks, kouter=False):
            p = k // NCHUNK
            s = st[k]
            wqk_t = wts[p]["wqk"]
            te.stage = "qkT"
            if kouter:
                for m0 in range(0, 16, 3):
                    ms = range(m0, min(m0 + 3, 16))
                    pqs = {m: ps_big.tile([128, CH], F32, tag="big",
                                          name=f"pq_{k}_{m}") for m in ms}
                    for kk in range(KB):
                        for m in ms:
                            te.matmul(
                                pqs[m][:],
                                lhsT=wqk_t[kk][:, m * 128:(m + 1) * 128],
                                rhs=s["xt"][kk][:],
                                start=(kk == 0), stop=(kk == KB - 1))
                    for m in ms:
                        nc.vector.tensor_copy(s["qkt"][m][:], pqs[m][:])
                return
            # m-pair order (j, 8+j): after pair j+1, head-pair j's q/k
            # tiles are ready, so its A pack weaves in -- one ~0.25us thin
            # pack per ~3.5us of fat keeps the HAM clock gate at 8/8.
            for pr in range(8):
                for m in (pr, pr + 8):
                    pq = ps_big.tile([128, CH], F32, tag="big",
                                     name=f"pq_{k}_{m}")
                    for kk in range(KB):
                        te.matmul(pq[:],
                                  lhsT=wqk_t[kk][:, m * 128:(m + 1) * 128],
                                  rhs=s["xt"][kk][:],
                                  start=(kk == 0), stop=(kk == KB - 1))
                    nc.vector.tensor_copy(s["qkt"][m][:], pq[:])
                    te.stage = "qkT"
                if pr >= 1:
                    emit_A(k, pr - 1)
                    te.stage = "qkT"
            emit_A(k, 7)

        def emit_y_stage(k):
            p, c = k // NCHUNK, k % NCHUNK
            s = st[k]
            wo_t = wts[p]["wo"]
            for tb in range(4):
                ysb = s["ybufs"][tb]
                for n2 in range(2):
                    te.stage = "y"
                    py = ps_big.tile([128, CH], F32, tag="big",
                                     name=f"py_{k}_{tb}_{n2}")
                    for i in range(KB):
                        kk = (i + tb * 2 + n2) % KB
                        te.matmul(
                            py[:],
                            lhsT=s["ot"][kk][:, tb * 128:(tb + 1) * 128],
                            rhs=wo_t[kk][:, n2 * 512:(n2 + 1) * 512],
                            start=(i == 0), stop=(i == KB - 1))
                    nc.vector.tensor_copy(
                        ysb[:, n2 * 512:(n2 + 1) * 512], py[:])
                    if n2 == 1:
                        if p == 1:
                            t0 = c * CH + tb * 128
                            nc.gpsimd.dma_start(
                                out[t0:t0 + 128, :], ysb[:],
                                accum_op=mybir.AluOpType.add)
                        else:
                            w0 = c * 8 + tb * 2
                            # issued inside the pack-free y stage, so
                            # the ~700ns issue cost delays no copies;
                            # keeps the xt queues (sync/gpsimd) clear
                            nc.scalar.dma_start(og[w0:w0 + 2, :, :], ysb[:])


        def new_state(k):
            st[k] = {
                "xt": None,
                "qkt": [p_qkt.tile([128, CH], BF16, tag="qkt",
                                   name=f"qkt_{k}_{i}") for i in range(16)],
                "v_t": [p_v.tile([128, D], BF16, tag="v",
                                 name=f"v_{k}_{i}") for i in range(4)],
                "sa": [None] * 8,
                "ot": [p_ot.tile([128, CH], BF16, tag="ot",
                                 name=f"ot_{k}_{i}") for i in range(8)],
                "ybufs": [p_y.tile([128, D], F32, tag="y",
                                   name=f"y_{k}_{tb}") for tb in range(4)],
            }

        # ---- prologue: weights pass 0 + chunk 0 interleaved; warmup ----
        xt0 = fetch_weights(0, interleave_xt0=True)
        te.stage = "warm"
        warm_ps = ps_big.tile([128, 128], F32, tag="big", name="warm_ps")
        for _ in range(16):
            te.matmul(warm_ps[:], lhsT=ident[:], rhs=ident[:],
                      start=True, stop=True)

        new_state(0)
        st[0]["xt"] = xt0
        xt_next = fetch_xt(1)

        # iteration 0: qkT(0) k-outer (starts as the weight tiles land),
        # then v(0), then A(0){0..7} as a block (the k-outer qkT can't
        # host the weave)
        emit_qkT_stage(0, [], kouter=True)
        emit_v_stage(0, [])
        te.stage = "att"
        for j in range(8):
            emit_A(0, j)

        for k in range(1, NIT + 1):
            last = k == NIT
            if not last:
                new_state(k)
                st[k]["xt"] = xt_next
                if k + 1 < NIT:
                    xt_next = fetch_xt(k + 1)
                if k % NCHUNK == NCHUNK - 1 and k // NCHUNK == 0:
                    fetch_weights(1)
                # qkT(k) hosts A(k){0..7} via the m-pair weave
                emit_qkT_stage(k, [])
                # v(k) hosts O(k-1){0..3}
                emit_v_stage(k, [(k - 1, j) for j in range(4)])
                # O(k-1){4..7}: short mini-block
                te.stage = "att"
                for j in range(4, 8):
                    emit_O(k - 1, j)
                # y(k-1): pack-free fat, lets the clock gate recover
                emit_y_stage(k - 1)
            else:
                # epilogue: finish chunk NIT-1
                te.stage = "att"
                for j in range(8):
                    emit_O(k - 1, j)
                emit_y_stage(k - 1)
            del st[k - 1]

    nc.compile()
    _BUILD_CACHE[key] = nc
    return nc


def _prep_inputs(x, w_qkv0, w_out0, w_qkv1, w_out1):
    bf = ml_dtypes.bfloat16
    x = np.asarray(x, dtype=np.float32)
    xth_all = np.ascontiguousarray(x.transpose(0, 3, 2, 1)
                                   .reshape(B, D, NT)).astype(bf)
    xtw_all = np.ascontiguousarray(x.transpose(0, 3, 1, 2)
                                   .reshape(B, D, NT)).astype(bf)
    common = {}
    for p, (wqkv, wout) in enumerate(((w_qkv0, w_out0), (w_qkv1, w_out1))):
        wqk_s = np.ascontiguousarray(wqkv[:, :2 * D]).copy()
        wqk_s[:, :D] *= SCALE
        common[f"wqk{p}"] = wqk_s.astype(bf)
        common[f"wv{p}"] = np.ascontiguousarray(wqkv[:, 2 * D:]).astype(bf)
        common[f"wo{p}"] = np.ascontiguousarray(wout).astype(bf)
    return [{"xth": xth_all[b], "xtw": xtw_all[b], **common}
            for b in range(B)]


def kernel(x, w_qkv0, w_out0, w_qkv1, w_out1, trace=False, tmpdir=None):
    nc = build()
    in_maps = _prep_inputs(x, w_qkv0, w_out0, w_qkv1, w_out1)
    res = run_bass_kernel_spmd(nc, in_maps, core_ids=list(range(B)),
                               trace=trace, tmpdir=tmpdir)
    outs = np.stack([res.results[b]["out"] for b in range(B)])
    outs = outs.reshape(B, 64, 64, D)
    kernel.last_result = res
    return outs
